# revision 12
# baseline (speedup 1.0000x reference)
"""GroupedQueryAttention TRN2 kernel: 8-way tensor-parallel over heads.

Sharding: core c gets query heads 4c..4c+3 (W_query rows 256c:256c+256),
KV head c (W_key/W_value rows 64c:64c+64), W_out columns 256c:256c+256.
x is replicated; each core computes a partial [C, T] output (transposed);
host transposes and sums.

All matmul operands are bf16 (1 PE cycle/row at any p-state and free size,
half the DMA bytes); PSUM accumulation stays f32.  Per-core dataflow:
  Stage 1 (per 512-col t-quarter, software-pipelined): xT streamed in,
    QKV projections (3 matmuls per 128-contraction chunk), PSUM->SBUF
    copies on ACT/DVE, RMS sumsq via PE ones-matmul + ACT sqrt + DVE
    recip, RoPE as rope_raw(q)*bcast(rinv) with the norm weights folded
    into per-dtype cos/sin tables (exact for any q/k_norm_w), v
    transposed into ones-augmented vaug via PE.
  Stage 2 attention per (head, 1024-col q-half): causal-trimmed S strips
    at 128 granularity, exp on ACT (scale=1/8 folded in) into bf16 P,
    triangle mask as bf16 multiply post-exp, A@V with ones-augmented V
    giving ctx + softmax sums in one accumulation; 1-ahead S pipeline
    against double-buffered PSUM.  Normalize via DVE recip + PE ones
    broadcast + DVE mul into bf16 ctxT.
  Stage 3 out-proj in [C, T] orientation (PSUM = [128 c-feat, 512 t]),
    copies alternate ACT/DVE, bf16 DMA out.
"""

import sys

sys.path.insert(0, "/opt/trn_rl_repo")

import numpy as np
import ml_dtypes

import concourse.bass as bass
import concourse.mybir as mybir
import concourse.tile as tile
from concourse import bacc
from concourse.bass_utils import run_bass_kernel_spmd

H, KV, D, EPS = 32, 8, 64, 1e-6
T = 2048
C = 2048
DQ = 256              # q out dims per core
NW = 512
F32 = mybir.dt.float32
BF16 = mybir.dt.bfloat16
AF = mybir.ActivationFunctionType
BF = ml_dtypes.bfloat16

_PROG = None


def _build_program():
    nc = bacc.Bacc("TRN2", target_bir_lowering=False, debug=False)

    xt_d = nc.declare_dram_parameter("xt", [C, T], BF16, isOutput=False)
    wq_d = nc.declare_dram_parameter("wq", [128, 16 * 384], BF16, isOutput=False)
    wo_d = nc.declare_dram_parameter("wo", [128, 2 * T], BF16, isOutput=False)
    cosq_d = nc.declare_dram_parameter("cosq", [128, T], BF16, isOutput=False)
    sinq_d = nc.declare_dram_parameter("sinq", [128, T], BF16, isOutput=False)
    cosk_d = nc.declare_dram_parameter("cosk", [64, T], BF16, isOutput=False)
    sink_d = nc.declare_dram_parameter("sink", [64, T], BF16, isOutput=False)
    tri_d = nc.declare_dram_parameter("tri", [128, 128], BF16, isOutput=False)
    sqo_d = nc.declare_dram_parameter("sqo", [128, 2], BF16, isOutput=False)
    perm_d = nc.declare_dram_parameter("perm", [128, 128], BF16, isOutput=False)
    id64_d = nc.declare_dram_parameter("id64", [64, 64], BF16, isOutput=False)
    onesv_d = nc.declare_dram_parameter("onesv", [128, 16], BF16, isOutput=False)
    bsel_d = nc.declare_dram_parameter("bsel", [65, 320], BF16, isOutput=False)
    bone_d = nc.declare_dram_parameter("bone", [1, 64], BF16, isOutput=False)
    out_d = nc.declare_dram_parameter("out", [C, T], BF16, isOutput=True)

    with tile.TileContext(nc) as tc:
        with tc.tile_pool(name="persist", bufs=1) as pp:
            qT = pp.tile([128, 2 * T], BF16, tag="qT")
            kkT = pp.tile([128, T], BF16, tag="kkT")
            vaug = pp.tile([128, 16 * 65], BF16, tag="vaug")
            ctxT = pp.tile([128, 2 * T], BF16, tag="ctxT")
            cosq = pp.tile([128, T], BF16, tag="cosq")
            sinq = pp.tile([128, T], BF16, tag="sinq")
            cosk = pp.tile([64, T], BF16, tag="cosk")
            sink = pp.tile([64, T], BF16, tag="sink")
            wq = pp.tile([128, 16 * 384], BF16, tag="wq")
            wo = pp.tile([128, 2 * T], BF16, tag="wo")
            tri = pp.tile([128, 128], BF16, tag="tri")
            sqo = pp.tile([128, 2], BF16, tag="sqo")
            perm = pp.tile([128, 128], BF16, tag="perm")
            id64 = pp.tile([64, 64], BF16, tag="id64")
            bsel = pp.tile([65, 320], BF16, tag="bsel")
            bone = pp.tile([1, 64], BF16, tag="bone")
            epsb = pp.tile([65, 1], F32, tag="epsb")
            nc.vector.memset(epsb[:], float(EPS))
            # Every ACT func used (Ln, Exp, Copy) lives in the
            # natural_log_exp_and_others table; leading with Ln keeps the
            # total table loads at 2 even if the scheduler interleaves.
            dsq = pp.tile([1, 1], F32, tag="dsq")
            nc.scalar.activation(dsq[:], epsb[0:1, :], AF.Ln,
                                 bias=epsb[0:1, :], scale=1.0)

            # ---------------- Stage 1: QKV + RMSNorm + RoPE ----------------
            with tc.tile_pool(name="s1x", bufs=2) as s1x, \
                 tc.tile_pool(name="s1s", bufs=2) as s1s, \
                 tc.tile_pool(name="s1pq", bufs=2, space="PSUM") as s1pq, \
                 tc.tile_pool(name="s1sm", bufs=2, space="PSUM") as s1sm:

                def xload(qtr):
                    xq = s1x.tile([128, 16 * NW], BF16, tag="xq")
                    src = xt_d[:, qtr * NW:(qtr + 1) * NW].rearrange(
                        "(c p) t -> p c t", p=128)
                    dst = xq[:].rearrange("p (c t) -> p c t", t=NW)
                    for g_ in range(4):
                        nc.sync.dma_start(dst[:, 4 * g_:4 * (g_ + 1), :],
                                          src[:, 4 * g_:4 * (g_ + 1), :])
                    return xq

                def qkv(qtr, xq=None):
                    if xq is None:
                        xq = xload(qtr)
                    pq0 = s1pq.tile([128, NW], F32, tag="pq0")
                    pq1 = s1pq.tile([128, NW], F32, tag="pq1")
                    pkv = s1pq.tile([128, NW], F32, tag="pkv")
                    for ci in range(16):
                        st, sp = ci == 0, ci == 15
                        xc = xq[:, NW * ci:NW * (ci + 1)]
                        nc.tensor.matmul(pq0[:], wq[:, 384 * ci:384 * ci + 128],
                                         xc, start=st, stop=sp)
                        nc.tensor.matmul(pq1[:],
                                         wq[:, 384 * ci + 128:384 * ci + 256],
                                         xc, start=st, stop=sp)
                        nc.tensor.matmul(pkv[:],
                                         wq[:, 384 * ci + 256:384 * ci + 384],
                                         xc, start=st, stop=sp)
                    return pq0, pq1, pkv

                def post(qtr, pq0, pq1, pkv):
                    w0 = qtr * NW
                    qraw0 = s1s.tile([128, NW], BF16, tag="qraw0")
                    qraw1 = s1s.tile([128, NW], BF16, tag="qraw1")
                    kraw = s1s.tile([64, NW], BF16, tag="kraw")
                    vraw = s1s.tile([64, NW], BF16, tag="vraw")
                    nc.scalar.copy(qraw0[:], pq0[:])
                    nc.scalar.copy(qraw1[:], pq1[:])
                    nc.vector.tensor_copy(kraw[:], pkv[0:64, :])
                    nc.vector.tensor_copy(vraw[:], pkv[64:128, :])
                    # PE: perm matmuls first (only need raw copies)
                    ppm0 = s1sm.tile([128, NW], F32, tag="sm")
                    nc.tensor.matmul(ppm0[:], perm[:], qraw0[:],
                                     start=True, stop=True)
                    ppm1 = s1sm.tile([128, NW], F32, tag="sm")
                    nc.tensor.matmul(ppm1[:], perm[:], qraw1[:],
                                     start=True, stop=True)
                    ppk = s1sm.tile([128, NW], F32, tag="sm")
                    nc.tensor.matmul(ppk[0:64, :], perm[0:64, 0:64], kraw[:],
                                     start=True, stop=True)
                    # RMS sumsq
                    t20 = s1s.tile([128, NW], BF16, tag="t20")
                    t21 = s1s.tile([128, NW], BF16, tag="t21")
                    t2k = s1s.tile([64, NW], BF16, tag="t2k")
                    nc.vector.tensor_mul(t20[:], qraw0[:], qraw0[:])
                    nc.vector.tensor_mul(t21[:], qraw1[:], qraw1[:])
                    nc.vector.tensor_mul(t2k[:], kraw[:], kraw[:])
                    srow = s1sm.tile([65, NW], F32, tag="sm")
                    nc.tensor.matmul(srow[0:2, :], sqo[:, 0:2], t20[:],
                                     start=True, stop=True)
                    nc.tensor.matmul(srow[32:34, :], sqo[:, 0:2], t21[:],
                                     start=True, stop=True)
                    nc.tensor.matmul(srow[64:65, :], sqo[0:64, 0:1], t2k[:],
                                     start=True, stop=True)
                    # RoPE partials that don't need rinv
                    tm1_0 = s1s.tile([128, NW], BF16, tag="tm1_0")
                    tm1_1 = s1s.tile([128, NW], BF16, tag="tm1_1")
                    tk1 = s1s.tile([64, NW], BF16, tag="tk1")
                    nc.vector.tensor_mul(tm1_0[:], qraw0[:], cosq[:, w0:w0 + NW])
                    nc.vector.tensor_mul(tm1_1[:], qraw1[:], cosq[:, w0:w0 + NW])
                    nc.vector.tensor_mul(tk1[:], kraw[:], cosk[:, w0:w0 + NW])
                    tsum0 = s1s.tile([128, NW], BF16, tag="tsum0")
                    tsum1 = s1s.tile([128, NW], BF16, tag="tsum1")
                    tks = s1s.tile([64, NW], BF16, tag="tks")
                    nc.vector.tensor_mul(tsum0[:], ppm0[:], sinq[:, w0:w0 + NW])
                    nc.vector.tensor_add(tsum0[:], tsum0[:], tm1_0[:])
                    nc.vector.tensor_mul(tsum1[:], ppm1[:], sinq[:, w0:w0 + NW])
                    nc.vector.tensor_add(tsum1[:], tsum1[:], tm1_1[:])
                    nc.vector.tensor_mul(tks[:], ppk[0:64, :], sink[:, w0:w0 + NW])
                    nc.vector.tensor_add(tks[:], tks[:], tk1[:])
                    # v -> vaug (transposed, ones-augmented)
                    for j in range(4):
                        ii = (w0 // 128) + j
                        pv = s1sm.tile([128, 64], BF16, tag="sm")
                        nc.tensor.transpose(pv[:], vraw[:, 128 * j:128 * (j + 1)],
                                            id64[:])
                        nc.vector.tensor_copy(vaug[:, 65 * ii:65 * ii + 64],
                                              pv[:])
                    # rinv = exp(-0.5 ln(var+eps)); single table family
                    lnv = s1s.tile([65, NW], F32, tag="rms5")
                    rb5 = s1s.tile([65, NW], BF16, tag="rb5")
                    for lo, hi in ((0, 2), (32, 34), (64, 65)):
                        nc.scalar.activation(lnv[lo:hi, :], srow[lo:hi, :],
                                             AF.Ln, bias=epsb[lo:hi, :],
                                             scale=1.0 / 64)
                        nc.scalar.activation(rb5[lo:hi, :], lnv[lo:hi, :],
                                             AF.Exp, scale=-0.5)
                    dsts = (qT[:, 0 * T + w0:0 * T + w0 + NW],
                            qT[:, 1 * T + w0:1 * T + w0 + NW],
                            kkT[0:64, w0:w0 + NW])
                    srcs = (tsum0, tsum1, tks)
                    for g, (plo, phi, lo, hi, npart) in enumerate(
                            ((0, 2, 0, 128, 128), (32, 34, 128, 256, 128),
                             (64, 65, 256, 320, 64))):
                        bb = s1sm.tile([128, NW], F32, tag="sm")
                        nc.tensor.matmul(bb[0:npart, :], bsel[plo:phi, lo:hi],
                                         rb5[plo:phi, :], start=True, stop=True)
                        nc.vector.tensor_mul(dsts[g], srcs[g][:],
                                             bb[0:npart, :])
                    nc.vector.tensor_copy(kkT[64:128, w0:w0 + NW],
                                          kkT[0:64, w0:w0 + NW])

                xq0 = s1x.tile([128, 16 * NW], BF16, tag="xq")
                src0 = xt_d[:, 0:NW].rearrange("(c p) t -> p c t", p=128)
                dst0 = xq0[:].rearrange("p (c t) -> p c t", t=NW)
                for g_ in range(4):
                    nc.sync.dma_start(wq[:, 1536 * g_:1536 * (g_ + 1)],
                                      wq_d[:, 1536 * g_:1536 * (g_ + 1)])
                    nc.sync.dma_start(dst0[:, 4 * g_:4 * (g_ + 1), :],
                                      src0[:, 4 * g_:4 * (g_ + 1), :])
                cur = qkv(0, xq0)
                xq1 = xload(1)
                for t_, d_ in ((cosq, cosq_d), (sinq, sinq_d), (cosk, cosk_d),
                               (sink, sink_d), (sqo, sqo_d), (perm, perm_d),
                               (id64, id64_d), (bsel, bsel_d), (tri, tri_d),
                               (bone, bone_d)):
                    nc.sync.dma_start(t_[:], d_[:])
                nc.sync.dma_start(
                    vaug[:].rearrange("p (i c) -> p i c", c=65)[:, :, 64:65],
                    onesv_d[:].rearrange("p (i c) -> p i c", c=1),
                )
                prev = cur
                cur = qkv(1, xq1)
                post(0, *prev)
                prev = cur
                cur = qkv(2)
                post(1, *prev)
                prev = cur
                cur = qkv(3)
                post(2, *prev)
                nc.sync.dma_start(wo[:], wo_d[:])
                post(3, *cur)

            # ------- Stage 2+3: attention (window-outer) + fused out-proj ----
            # ctx = [65, 512] per (head, 512-col q-window); out-proj for
            # window w interleaves into window w+1's attention stream.
            with tc.tile_pool(name="actx", bufs=2, space="PSUM") as actx, \
                 tc.tile_pool(name="asp", bufs=3, space="PSUM") as asp, \
                 tc.tile_pool(name="ops", bufs=3, space="PSUM") as ops, \
                 tc.tile_pool(name="aptp", bufs=3) as aptp, \
                 tc.tile_pool(name="asb", bufs=2) as asb, \
                 tc.tile_pool(name="osb", bufs=4) as osb:

                def s_of(i, qrow, m, w):
                    col0 = max(128 * i - 512 * w, 0)     # window-relative
                    st = asp.tile([128, 512], F32, tag="s")
                    nc.tensor.matmul(
                        st[:, col0:512],
                        kkT[qrow:qrow + 64, 128 * i:128 * (i + 1)],
                        qT[qrow:qrow + 64,
                           m * T + 512 * w + col0:m * T + 512 * (w + 1)],
                        start=True, stop=True)
                    return st, col0

                def do_norm(ctx, qrow, m, w):
                    rb = asb.tile([1, 512], BF16, tag="rb")
                    with nc.allow_low_precision(reason="softmax recip"):
                        nc.vector.reciprocal(rb[:], ctx[64:65, :])
                    rbbp = asp.tile([64, 512], F32, tag="s")
                    nc.tensor.matmul(rbbp[:], bone[:], rb[:],
                                     start=True, stop=True)
                    rbbs = asb.tile([64, 512], BF16, tag="rbbs")
                    nc.vector.tensor_copy(rbbs[:], rbbp[:])
                    nc.vector.tensor_mul(
                        ctxT[qrow:qrow + 64, m * T + 512 * w:m * T + 512 * (w + 1)],
                        ctx[0:64, :], rbbs[:])

                def outproj(tw, cbs):
                    for cb in cbs:
                        po = ops.tile([128, 512], F32, tag="po")
                        for mm in range(2):
                            nc.tensor.matmul(
                                po[:],
                                wo[:, T * mm + 128 * cb:T * mm + 128 * (cb + 1)],
                                ctxT[:, T * mm + 512 * tw:T * mm + 512 * (tw + 1)],
                                start=(mm == 0), stop=(mm == 1))
                        ob = osb.tile([128, 512], BF16, tag="ob")
                        if cb % 2 == 0:
                            nc.scalar.copy(ob[:], po[:])
                        else:
                            nc.vector.tensor_copy(ob[:], po[:])
                        nc.sync.dma_start(
                            out_d[128 * cb:128 * (cb + 1),
                                  512 * tw:512 * (tw + 1)], ob[:])

                norm_pend = None
                for w in range(4):
                    for h in range(4):
                        sub, m = h % 2, h // 2
                        qrow = 64 * sub
                        nstrips = 4 * w + 4
                        ctx = actx.tile([65, 512], F32, tag="ctx")
                        pend = s_of(0, qrow, m, w)
                        if norm_pend is not None:
                            do_norm(*norm_pend)
                            norm_pend = None
                        for i in range(nstrips):
                            st, col0 = pend
                            pt = aptp.tile([128, 512], BF16, tag="pt")
                            nc.scalar.activation(pt[:, col0:512], st[:, col0:512],
                                                 AF.Exp, scale=0.125)
                            if i >= 4 * w:
                                nc.vector.tensor_mul(
                                    pt[:, col0:col0 + 128],
                                    pt[:, col0:col0 + 128], tri[:])
                            if i + 1 < nstrips:
                                pend = s_of(i + 1, qrow, m, w)
                            if col0 > 0:
                                nc.vector.memset(pt[:, 0:col0], 0.0)
                            nc.tensor.matmul(
                                ctx[:], vaug[:, 65 * i:65 * (i + 1)], pt[:],
                                start=(i == 0), stop=(i == nstrips - 1))
                        norm_pend = (ctx, qrow, m, w)
                        if w > 0:
                            outproj(w - 1, range(4 * h, 4 * h + 4))
                do_norm(*norm_pend)
                outproj(3, range(16))

    nc.compile()
    return nc


def kernel(x, mask, cos, sin, W_query, W_key, W_value, W_out,
           q_norm_w, k_norm_w):
    global _PROG
    if _PROG is None:
        _PROG = _build_program()
    nc = _PROG

    x = np.asarray(x, np.float32)
    cos = np.asarray(cos, np.float32)
    sin = np.asarray(sin, np.float32)
    W_query = np.asarray(W_query, np.float32)
    W_key = np.asarray(W_key, np.float32)
    W_value = np.asarray(W_value, np.float32)
    W_out = np.asarray(W_out, np.float32)
    q_norm_w = np.asarray(q_norm_w, np.float32)
    k_norm_w = np.asarray(k_norm_w, np.float32)

    xt = np.ascontiguousarray(x[0].T).astype(BF)            # [C, T]

    # RoPE tables with the RMSNorm weights folded in:
    #   rope(q * w)[d] = q[d] w[d] cos[d] + sign[d] q[d^32] w[d^32] sin[d]
    cos1 = cos[:T].T.astype(np.float32)                     # [64, T]
    sin1 = sin[:T].T.astype(np.float32).copy()
    sgn = np.where((np.arange(D) % 64) < 32, -1.0, 1.0).astype(np.float32)
    dperm = np.arange(D) ^ 32
    cq1 = cos1 * q_norm_w[:, None]
    sq1 = sin1 * sgn[:, None] * q_norm_w[dperm][:, None]
    cosq = np.ascontiguousarray(np.concatenate([cq1, cq1], 0)).astype(BF)
    sinq = np.ascontiguousarray(np.concatenate([sq1, sq1], 0)).astype(BF)
    cosk = np.ascontiguousarray(cos1 * k_norm_w[:, None]).astype(BF)
    sink = np.ascontiguousarray(
        sin1 * sgn[:, None] * k_norm_w[dperm][:, None]).astype(BF)

    p = np.arange(128)[:, None]
    j = np.arange(128)[None, :]
    tri = np.where(p > j, 0.0, 1.0).astype(BF)              # S^T triangle

    sqo = np.zeros((128, 2), np.float32)
    sqo[0:64, 0] = 1.0
    sqo[64:128, 1] = 1.0
    perm = np.zeros((128, 128), np.float32)
    for b in range(2):
        for d_ in range(64):
            perm[64 * b + (d_ ^ 32), 64 * b + d_] = 1.0
    bsel = np.zeros((65, 320), np.float32)
    bsel[0, 0:64] = 1.0
    bsel[1, 64:128] = 1.0
    bsel[32, 128 + 0:128 + 64] = 1.0
    bsel[33, 128 + 64:128 + 128] = 1.0
    bsel[64, 256:320] = 1.0

    shared = {
        "xt": xt, "cosq": cosq, "sinq": sinq, "cosk": cosk, "sink": sink,
        "tri": tri, "sqo": sqo.astype(BF), "perm": perm.astype(BF),
        "id64": np.eye(64, dtype=np.float32).astype(BF),
        "onesv": np.ones((128, 16), np.float32).astype(BF),
        "bsel": bsel.astype(BF), "bone": np.ones((1, 64), np.float32).astype(BF),
    }
    in_maps = []
    for c in range(8):
        wqkv = np.concatenate(
            [W_query[DQ * c:DQ * (c + 1)],
             W_key[64 * c:64 * (c + 1)],
             W_value[64 * c:64 * (c + 1)]], axis=0).T       # [C, 384]
        wq_h = np.ascontiguousarray(
            wqkv.reshape(16, 128, 384).transpose(1, 0, 2).reshape(128, 6144)
        ).astype(BF)
        wo_sl = W_out[:, DQ * c:DQ * (c + 1)].T             # [256, C]
        wo_h = np.ascontiguousarray(
            wo_sl.reshape(2, 128, C).transpose(1, 0, 2).reshape(128, 2 * C)
        ).astype(BF)
        in_maps.append(dict(shared, wq=wq_h, wo=wo_h))

    res = run_bass_kernel_spmd(nc, in_maps, list(range(8)))
    acc = np.zeros((C, T), np.float32)
    for c in range(8):
        acc += res.results[c]["out"].astype(np.float32)
    return np.ascontiguousarray(acc.T)[None]
